# revision 29
# baseline (speedup 1.0000x reference)
# Grouped-GEMM MoE (8 experts, top-2, SwiGLU) on 8 Trainium2 NeuronCores.
#
# Strategy (expert-parallel, host-side all-to-all):
#   - Host routes token-slots to experts (argsort of top_experts), builds a
#     padded, pre-tiled, transposed activation matrix X.T per expert, split
#     into NCH column chunks (widths WS, each <= 512 for PSUM/moving-operand
#     limits and >= 272 so LDWEIGHTS always hides under the matmul stream).
#   - Core e runs expert e's dense MLP feature-major: every matmul contracts
#     over the partition dim, the token dim always rides the free dim:
#         H.T  = W1.T-tiles @ X.T      (bf16, K=2048)
#         actT = silu(Ha + b1a) * (Hb + b1b)   (ACT + DVE, fused bias)
#         Y.T  = W2.T-tiles @ actT     (bf16, K=2816)
#   - Host scatters Y rows back to token slots and does the weighted top-k
#     combine.
#
# Startup pipelining: the first FPA f-tile-pairs of GEMM1 process only
# chunk 0 (one ~2MB xt DMA + modest weight bandwidth), buying time for the
# remaining xt chunks to stream in; the deferred chunk-1.. work is
# interleaved into the remaining f-tile-pairs with re-fetched weight strips.
# Weight fills ride the sync queue (not scalar, which is busy with ACTs).

import os

import ml_dtypes
import numpy as np

import concourse.bacc as bacc
import concourse.mybir as mybir
import concourse.tile as tile
from concourse import bass_utils

P = 128
MINW = 272        # min chunk width: LDWEIGHTS (107ns) < W/2.4GHz stream time
MAXW = 512        # PSUM bank / moving-operand limit (fp32)
C_MAX = 1536      # max token capacity per wave (SBUF-resident xt + act)
FPA = 8           # f-tile-pairs processed chunk-0-only during xt streaming

f32 = mybir.dt.float32
bf16 = mybir.dt.bfloat16
Silu = mybir.ActivationFunctionType.Silu
Identity = mybir.ActivationFunctionType.Identity
Alu = mybir.AluOpType

_cache = {}

# set by the most recent kernel() call when KERNEL_TRACE=1 (test harness use)
last_exec_time_ns = None
last_results = None


def _ensure_trace_hooks():
    """Install the NTFF profile hook shim if antenv.axon_hooks is absent.

    The container's antenv is a stub without axon_hooks, so trace=True in
    run_bass_kernel_spmd would otherwise crash. Build the hook from
    trn_agent_boot's ctypes helper and register it under the module name
    bass_utils imports. Also neuter the network artifact upload. Best
    effort: any failure leaves tracing degraded but the kernel running.
    """
    import sys
    import types

    try:
        bass_utils.upload_artifacts = lambda tmpdir: "local://" + tmpdir
        try:
            import antenv.axon_hooks  # noqa: F401
            return
        except ImportError:
            pass
        from trn_agent_boot.trn_boot import _ntff_profile_via_ctypes

        hook = _ntff_profile_via_ctypes("/opt/axon/libaxon_pjrt.so")
        m = types.ModuleType("antenv.axon_hooks")
        m.get_axon_ntff_profile_hook = lambda: hook
        m.set_axon_ntff_profile_hook = lambda h: None
        import antenv  # noqa: F401
        sys.modules["antenv.axon_hooks"] = m
    except Exception:
        pass


def _widths(cmax):
    """Chunk widths: multiples of 8 in [MINW, MAXW], minimal total >= cmax."""
    cmax = max(int(cmax), 1)
    nch = max(1, -(-cmax // MAXW))
    cap = max(MINW * nch, ((cmax + 7) // 8) * 8)
    ws = []
    rem = cap - MINW * nch
    for _ in range(nch):
        take = min(MAXW - MINW, rem)
        ws.append(MINW + take)
        rem -= take
    assert rem == 0 and sum(ws) == cap
    return ws


def _build(ws, fpa, H, F, n_cores):
    """Build+schedule the per-core MLP program for chunk widths ws."""
    ws = list(ws)
    NCH = len(ws)
    C = sum(ws)
    offs = [sum(ws[:i]) for i in range(NCH)]
    F2 = 2 * F
    KT1 = H // P      # k-tiles of GEMM1 (16)
    FT = F2 // P      # f-tiles of W1    (44)
    FP = F // P       # f-tile pairs / k-tiles of GEMM2 (22)
    MT = H // P       # m-tiles of GEMM2 (16)

    nc = bacc.Bacc("TRN2", target_bir_lowering=False, debug=False,
                   num_devices=n_cores)

    # pre-tiled inputs: xt{c}[p, k*W + w] = x.T[k*P + p, offs[c] + w]
    xt_d = [nc.dram_tensor(f"xt{c}", (P, KT1 * ws[c]), bf16,
                           kind="ExternalInput").ap() for c in range(NCH)]
    # w1t[ft, p, ko, fi] = w1[ko*P + p, ft*P + fi]
    w1_d = nc.dram_tensor("w1t", (FT, P, KT1, P), bf16, kind="ExternalInput").ap()
    b1_d = nc.dram_tensor("b1", (F2, 1), f32, kind="ExternalInput").ap()
    # w2t[mt, p, ko, mi] = w2[ko*P + p, mt*P + mi]
    w2_d = nc.dram_tensor("w2t", (MT, P, FP, P), bf16, kind="ExternalInput").ap()
    b2_d = nc.dram_tensor("b2", (H, 1), f32, kind="ExternalInput").ap()
    # yt[m, p, c] = y.T[m*P + p, c]
    yt_d = nc.dram_tensor("yt", (MT, P, C), f32, kind="ExternalOutput").ap()

    xt_t = [xt_d[c].rearrange("p (k w) -> p k w", w=ws[c]) for c in range(NCH)]
    b1_t = b1_d.rearrange("(ft p) one -> p ft one", p=P)
    b2_t = b2_d.rearrange("(mt p) one -> p mt one", p=P)

    with tile.TileContext(nc) as tc:
        with tc.tile_pool(name="persist", bufs=1) as persist, \
             tc.tile_pool(name="w1pool", bufs=8) as w1pool, \
             tc.tile_pool(name="w2pool", bufs=3) as w2pool, \
             tc.tile_pool(name="spool", bufs=4) as spool, \
             tc.tile_pool(name="ypool", bufs=2) as ypool, \
             tc.tile_pool(name="psum", bufs=8, space="PSUM") as psum:

            # PE warm-up: zero-matmuls with no DMA deps keep the HAM activity
            # window busy during the input-DMA ramp, so real matmuls start at
            # 2.4GHz instead of the cold 1.2GHz. Emitted first on otherwise
            # idle queues (vector memset) so nothing delays them.
            warm_sb = persist.tile([P, MAXW], bf16, tag="warm")
            nc.vector.memset(warm_sb[:], 0)
            wps = psum.tile([P, MAXW], f32, tag="ps", name="warm_ps")
            for _ in range(15):
                nc.tensor.matmul(wps, warm_sb[:, :P], warm_sb[:],
                                 start=True, stop=True)

            # xt chunks: ch0 first (split along k for an earlier first matmul);
            # later chunks are gated behind pass-A progress (delay anchors) so
            # they don't steal HBM bandwidth from the startup-critical w1
            # pairs — they aren't consumed until the backlog entries anyway.
            xt_sb = [persist.tile([P, KT1, ws[c]], bf16, tag=f"xt{c}",
                                  name=f"xt_sb{c}")
                     for c in range(NCH)]
            q = KT1 // 4
            for s4 in range(4):
                nc.gpsimd.dma_start(xt_sb[0][:, s4 * q:(s4 + 1) * q],
                                    xt_t[0][:, s4 * q:(s4 + 1) * q])
            for c in range(1, NCH):
                nc.gpsimd.dma_start(xt_sb[c][:], xt_t[c])

            act_sb = persist.tile([P, FP, C], bf16)
            b1_sb = persist.tile([P, FT, 1], f32)
            nc.scalar.dma_start(b1_sb[:], b1_t)
            b2_sb = persist.tile([P, MT, 1], f32)
            nc.scalar.dma_start(b2_sb[:], b2_t)

            def g1_entry(fp, chunks, strips=None):
                if strips is None:
                    w1a = w1pool.tile([P, KT1, P], bf16, tag="w1s")
                    w1b = w1pool.tile([P, KT1, P], bf16, tag="w1s")
                    nc.sync.dma_start(w1a[:], w1_d[fp])
                    nc.sync.dma_start(w1b[:], w1_d[FP + fp])
                else:
                    w1a, w1b = strips
                pas = [psum.tile([P, ws[c]], f32, tag="ps", name=f"pa{c}")
                       for c in chunks]
                for k in range(KT1):
                    for i, c in enumerate(chunks):
                        nc.tensor.matmul(pas[i], w1a[:, k], xt_sb[c][:, k],
                                         start=(k == 0), stop=(k == KT1 - 1))
                sils = []
                for i, c in enumerate(chunks):
                    s = spool.tile([P, ws[c]], f32, tag="s")
                    nc.scalar.activation(s, pas[i], Silu, bias=b1_sb[:, fp])
                    sils.append(s)
                pbs = [psum.tile([P, ws[c]], f32, tag="ps", name=f"pb{c}")
                       for c in chunks]
                for k in range(KT1):
                    for i, c in enumerate(chunks):
                        nc.tensor.matmul(pbs[i], w1b[:, k], xt_sb[c][:, k],
                                         start=(k == 0), stop=(k == KT1 - 1))
                # act = (pb + b1b) * silu(pa + b1a), cast bf16 on write
                for i, c in enumerate(chunks):
                    nc.vector.scalar_tensor_tensor(
                        act_sb[:, fp, offs[c]:offs[c] + ws[c]], pbs[i],
                        b1_sb[:, FP + fp], sils[i], Alu.add, Alu.mult)

            # ---- GEMM1 schedule: chunk-0-only head, backlog interleaved
            schedule = []
            if NCH >= 2 and fpa > 0:
                head = min(fpa, FP)
                for fp in range(head):
                    schedule.append((fp, [0]))
                backlog = list(range(head))
                rest = list(range(1, NCH))
                for fp in range(head, FP):
                    schedule.append((fp, list(range(NCH))))
                    if backlog:
                        schedule.append((backlog.pop(0), rest))
                while backlog:
                    schedule.append((backlog.pop(0), rest))
            else:
                for fp in range(FP):
                    schedule.append((fp, list(range(NCH))))

            with nc.named_scope("gemm1"):
                for fp, chunks in schedule:
                    g1_entry(fp, chunks)

            # ---- GEMM2: k-outer over all chunks per weight tile
            with nc.named_scope("gemm2"):
                for m in range(MT):
                    w2s = w2pool.tile([P, FP, P], bf16, tag="w2s")
                    nc.sync.dma_start(w2s[:], w2_d[m])
                    pys = [psum.tile([P, ws[c]], f32, tag="ps", name=f"py{c}")
                           for c in range(NCH)]
                    for k in range(FP):
                        for c in range(NCH):
                            nc.tensor.matmul(
                                pys[c], w2s[:, k],
                                act_sb[:, k, offs[c]:offs[c] + ws[c]],
                                start=(k == 0), stop=(k == FP - 1))
                    y = ypool.tile([P, C], f32, tag="y")
                    for c in range(NCH):
                        ysl = y[:, offs[c]:offs[c] + ws[c]]
                        dst = yt_d[m][:, offs[c]:offs[c] + ws[c]]
                        # evac + store ride the same engine per chunk so the
                        # three chains drain in parallel at the kernel tail
                        if c % 2 == 0:
                            nc.scalar.activation(ysl, pys[c], Identity,
                                                 bias=b2_sb[:, m])
                            eng = nc.scalar if c == 0 else nc.sync
                        else:
                            nc.vector.tensor_scalar_add(ysl, pys[c],
                                                        b2_sb[:, m])
                            eng = nc.sync
                        eng.dma_start(dst, ysl)

    nc.compile()
    return nc


def kernel(hidden_states, expert_weights, w1, b1, w2, b2, top_experts):
    global last_exec_time_ns, last_results

    hidden_states = np.asarray(hidden_states)
    B, S, H = hidden_states.shape
    E, _, F2 = np.asarray(w1).shape
    F = F2 // 2
    topk = np.asarray(top_experts).shape[-1]
    N = B * S
    n_cores = 8
    assert E == n_cores, f"kernel assumes one expert per core, got E={E}"
    KT1 = H // P

    x = np.ascontiguousarray(hidden_states.reshape(N, H).astype(np.float32))
    te = np.asarray(top_experts).reshape(-1).astype(np.int64)
    ew = np.asarray(expert_weights).reshape(-1).astype(np.float32)

    counts = np.bincount(te, minlength=E)
    order = np.argsort(te, kind="stable")      # slot ids grouped by expert
    starts = np.zeros(E + 1, dtype=np.int64)
    starts[1:] = np.cumsum(counts)

    cmax = int(counts.max())
    ws = _widths(min(max(cmax, 1), C_MAX))
    cap = sum(ws)
    offs = [sum(ws[:i]) for i in range(len(ws))]
    n_waves = max(1, -(-cmax // cap))
    fpa = FPA if len(ws) >= 2 else 0

    key = (tuple(ws), fpa, H, F, n_cores)
    if key not in _cache:
        _cache[key] = _build(*key)
    nc = _cache[key]

    # per-expert constant inputs (weights pre-tiled to contiguous SBUF strips)
    FT, FP, MT = F2 // P, F // P, H // P
    const_maps = []
    for e in range(E):
        w1e = np.asarray(w1[e], dtype=np.float32).astype(ml_dtypes.bfloat16)
        w1t = np.ascontiguousarray(
            w1e.reshape(KT1, P, FT, P).transpose(2, 1, 0, 3))
        w2e = np.asarray(w2[e], dtype=np.float32).astype(ml_dtypes.bfloat16)
        w2t = np.ascontiguousarray(
            w2e.reshape(FP, P, MT, P).transpose(2, 1, 0, 3))
        const_maps.append({
            "w1t": w1t,
            "b1": np.ascontiguousarray(
                np.asarray(b1[e], dtype=np.float32).reshape(F2, 1)),
            "w2t": w2t,
            "b2": np.ascontiguousarray(
                np.asarray(b2[e], dtype=np.float32).reshape(H, 1)),
        })

    trace = os.environ.get("KERNEL_TRACE", "") == "1"
    if trace:
        _ensure_trace_hooks()
    out_pairs = np.zeros((N * topk, H), dtype=np.float32)
    last_results = []
    for w in range(n_waves):
        in_maps = []
        for e in range(E):
            lo = w * cap
            idx = order[starts[e] + lo: min(starts[e + 1], starts[e] + lo + cap)]
            toks = idx // topk
            xp = np.zeros((cap, H), dtype=ml_dtypes.bfloat16)
            if len(toks):
                xp[:len(toks)] = x[toks]
            # [cap, KT1, P] -> [P, KT1, cap] -> per-chunk contiguous strips
            arr = xp.reshape(cap, KT1, P).transpose(2, 1, 0)
            im = {f"xt{c}": np.ascontiguousarray(
                      arr[:, :, offs[c]:offs[c] + ws[c]]).reshape(P, -1)
                  for c in range(len(ws))}
            im.update(const_maps[e])
            in_maps.append(im)
        tmpdir = None
        if trace:
            import shutil
            tmpdir = f"/tmp/moe_trace_w{w}"
            shutil.rmtree(tmpdir, ignore_errors=True)
            os.makedirs(tmpdir, exist_ok=True)
        res = bass_utils.run_bass_kernel_spmd(
            nc, in_maps, core_ids=list(range(n_cores)), trace=trace,
            tmpdir=tmpdir)
        last_results.append(res)
        if trace:
            last_exec_time_ns = res.exec_time_ns
        for e in range(E):
            lo = w * cap
            idx = order[starts[e] + lo: min(starts[e + 1], starts[e] + lo + cap)]
            if len(idx):
                yt = np.asarray(res.results[e]["yt"],
                                dtype=np.float32).reshape(H, cap)
                out_pairs[idx] = yt[:, :len(idx)].T

    out = (out_pairs.reshape(N, topk, H) * ew.reshape(N, topk, 1)).sum(axis=1)
    return out.reshape(B, S, H).astype(np.float32)


# revision 32
# speedup vs baseline: 1.0048x; 1.0048x over previous
# Grouped-GEMM MoE (8 experts, top-2, SwiGLU) on 8 Trainium2 NeuronCores.
#
# Strategy (expert-parallel, host-side all-to-all):
#   - Host routes token-slots to experts (argsort of top_experts), builds a
#     padded, pre-tiled, transposed activation matrix X.T per expert, split
#     into NCH column chunks (widths WS, each <= 512 for PSUM/moving-operand
#     limits and >= 272 so LDWEIGHTS always hides under the matmul stream).
#   - Core e runs expert e's dense MLP feature-major: every matmul contracts
#     over the partition dim, the token dim always rides the free dim:
#         H.T  = W1.T-tiles @ X.T      (bf16, K=2048)
#         actT = silu(Ha + b1a) * (Hb + b1b)   (ACT + DVE, fused bias)
#         Y.T  = W2.T-tiles @ actT     (bf16, K=2816)
#   - Host scatters Y rows back to token slots and does the weighted top-k
#     combine.
#
# Startup pipelining: the first FPA f-tile-pairs of GEMM1 process only
# chunk 0 (one ~2MB xt DMA + modest weight bandwidth), buying time for the
# remaining xt chunks to stream in; the deferred chunk-1.. work is
# interleaved into the remaining f-tile-pairs with re-fetched weight strips.
# Weight fills ride the sync queue (not scalar, which is busy with ACTs).

import os

import ml_dtypes
import numpy as np

import concourse.bacc as bacc
import concourse.mybir as mybir
import concourse.tile as tile
from concourse import bass_utils

P = 128
MINW = 272        # min chunk width: LDWEIGHTS (107ns) < W/2.4GHz stream time
MAXW = 512        # PSUM bank / moving-operand limit (fp32)
C_MAX = 1536      # max token capacity per wave (SBUF-resident xt + act)
FPA = 8           # f-tile-pairs processed chunk-0-only during xt streaming

f32 = mybir.dt.float32
bf16 = mybir.dt.bfloat16
Silu = mybir.ActivationFunctionType.Silu
Identity = mybir.ActivationFunctionType.Identity
Alu = mybir.AluOpType

_cache = {}

# set by the most recent kernel() call when KERNEL_TRACE=1 (test harness use)
last_exec_time_ns = None
last_results = None


def _ensure_trace_hooks():
    """Install the NTFF profile hook shim if antenv.axon_hooks is absent.

    The container's antenv is a stub without axon_hooks, so trace=True in
    run_bass_kernel_spmd would otherwise crash. Build the hook from
    trn_agent_boot's ctypes helper and register it under the module name
    bass_utils imports. Also neuter the network artifact upload. Best
    effort: any failure leaves tracing degraded but the kernel running.
    """
    import sys
    import types

    try:
        bass_utils.upload_artifacts = lambda tmpdir: "local://" + tmpdir
        try:
            import antenv.axon_hooks  # noqa: F401
            return
        except ImportError:
            pass
        from trn_agent_boot.trn_boot import _ntff_profile_via_ctypes

        hook = _ntff_profile_via_ctypes("/opt/axon/libaxon_pjrt.so")
        m = types.ModuleType("antenv.axon_hooks")
        m.get_axon_ntff_profile_hook = lambda: hook
        m.set_axon_ntff_profile_hook = lambda h: None
        import antenv  # noqa: F401
        sys.modules["antenv.axon_hooks"] = m
    except Exception:
        pass


def _widths(cmax):
    """Chunk widths: multiples of 8 in [MINW, MAXW], minimal total >= cmax."""
    cmax = max(int(cmax), 1)
    nch = max(1, -(-cmax // MAXW))
    cap = max(MINW * nch, ((cmax + 7) // 8) * 8)
    ws = []
    rem = cap - MINW * nch
    for _ in range(nch):
        take = min(MAXW - MINW, rem)
        ws.append(MINW + take)
        rem -= take
    assert rem == 0 and sum(ws) == cap
    return ws


def _build(ws, fpa, H, F, n_cores):
    """Build+schedule the per-core MLP program for chunk widths ws."""
    ws = list(ws)
    NCH = len(ws)
    C = sum(ws)
    offs = [sum(ws[:i]) for i in range(NCH)]
    F2 = 2 * F
    KT1 = H // P      # k-tiles of GEMM1 (16)
    FT = F2 // P      # f-tiles of W1    (44)
    FP = F // P       # f-tile pairs / k-tiles of GEMM2 (22)
    MT = H // P       # m-tiles of GEMM2 (16)

    nc = bacc.Bacc("TRN2", target_bir_lowering=False, debug=False,
                   num_devices=n_cores)

    # pre-tiled inputs: xt{c}[p, k*W + w] = x.T[k*P + p, offs[c] + w]
    xt_d = [nc.dram_tensor(f"xt{c}", (P, KT1 * ws[c]), bf16,
                           kind="ExternalInput").ap() for c in range(NCH)]
    # w1t[ft, p, ko, fi] = w1[ko*P + p, ft*P + fi]
    w1_d = nc.dram_tensor("w1t", (FT, P, KT1, P), bf16, kind="ExternalInput").ap()
    b1_d = nc.dram_tensor("b1", (F2, 1), f32, kind="ExternalInput").ap()
    # w2t[mt, p, ko, mi] = w2[ko*P + p, mt*P + mi]
    w2_d = nc.dram_tensor("w2t", (MT, P, FP, P), bf16, kind="ExternalInput").ap()
    b2_d = nc.dram_tensor("b2", (H, 1), f32, kind="ExternalInput").ap()
    # yt[m, p, c] = y.T[m*P + p, c]
    yt_d = nc.dram_tensor("yt", (MT, P, C), f32, kind="ExternalOutput").ap()

    xt_t = [xt_d[c].rearrange("p (k w) -> p k w", w=ws[c]) for c in range(NCH)]
    b1_t = b1_d.rearrange("(ft p) one -> p ft one", p=P)
    b2_t = b2_d.rearrange("(mt p) one -> p mt one", p=P)

    with tile.TileContext(nc) as tc:
        with tc.tile_pool(name="persist", bufs=1) as persist, \
             tc.tile_pool(name="wpool", bufs=8) as wpool, \
             tc.tile_pool(name="epool", bufs=4) as epool, \
             tc.tile_pool(name="psum", bufs=8, space="PSUM") as psum:

            # PE warm-up: zero-matmuls with no DMA deps keep the HAM activity
            # window busy during the input-DMA ramp, so real matmuls start at
            # 2.4GHz instead of the cold 1.2GHz. Emitted first on otherwise
            # idle queues (vector memset) so nothing delays them.
            warm_sb = persist.tile([P, MAXW], bf16, tag="warm")
            nc.vector.memset(warm_sb[:], 0)
            wps = psum.tile([P, MAXW], f32, tag="ps", name="warm_ps")
            for _ in range(15):
                nc.tensor.matmul(wps, warm_sb[:, :P], warm_sb[:],
                                 start=True, stop=True)

            # xt chunks: ch0 first (split along k for an earlier first matmul);
            # later chunks are gated behind pass-A progress (delay anchors) so
            # they don't steal HBM bandwidth from the startup-critical w1
            # pairs — they aren't consumed until the backlog entries anyway.
            xt_sb = [persist.tile([P, KT1, ws[c]], bf16, tag=f"xt{c}",
                                  name=f"xt_sb{c}")
                     for c in range(NCH)]
            q = KT1 // 4
            for s4 in range(4):
                nc.gpsimd.dma_start(xt_sb[0][:, s4 * q:(s4 + 1) * q],
                                    xt_t[0][:, s4 * q:(s4 + 1) * q])
            for c in range(1, NCH):
                nc.gpsimd.dma_start(xt_sb[c][:], xt_t[c])

            act_sb = persist.tile([P, FP, C], bf16)
            b1_sb = persist.tile([P, FT, 1], f32)
            nc.scalar.dma_start(b1_sb[:], b1_t)
            b2_sb = persist.tile([P, MT, 1], f32)
            nc.scalar.dma_start(b2_sb[:], b2_t)

            def g1_entry(fp, chunks, strips=None):
                if strips is None:
                    w1a = wpool.tile([P, KT1, P], bf16, tag="w1s")
                    w1b = wpool.tile([P, KT1, P], bf16, tag="w1s")
                    nc.sync.dma_start(w1a[:], w1_d[fp])
                    nc.sync.dma_start(w1b[:], w1_d[FP + fp])
                else:
                    w1a, w1b = strips
                pas = [psum.tile([P, ws[c]], f32, tag="ps", name=f"pa{c}")
                       for c in chunks]
                for k in range(KT1):
                    for i, c in enumerate(chunks):
                        nc.tensor.matmul(pas[i], w1a[:, k], xt_sb[c][:, k],
                                         start=(k == 0), stop=(k == KT1 - 1))
                sils = []
                for i, c in enumerate(chunks):
                    s = epool.tile([P, ws[c]], f32, tag="s")
                    nc.scalar.activation(s, pas[i], Silu, bias=b1_sb[:, fp])
                    sils.append(s)
                pbs = [psum.tile([P, ws[c]], f32, tag="ps", name=f"pb{c}")
                       for c in chunks]
                for k in range(KT1):
                    for i, c in enumerate(chunks):
                        nc.tensor.matmul(pbs[i], w1b[:, k], xt_sb[c][:, k],
                                         start=(k == 0), stop=(k == KT1 - 1))
                # act = (pb + b1b) * silu(pa + b1a), cast bf16 on write
                for i, c in enumerate(chunks):
                    nc.vector.scalar_tensor_tensor(
                        act_sb[:, fp, offs[c]:offs[c] + ws[c]], pbs[i],
                        b1_sb[:, FP + fp], sils[i], Alu.add, Alu.mult)

            # ---- GEMM1 schedule: chunk-0-only head, backlog interleaved
            schedule = []
            if NCH >= 2 and fpa > 0:
                head = min(fpa, FP)
                for fp in range(head):
                    schedule.append((fp, [0]))
                backlog = list(range(head))
                rest = list(range(1, NCH))
                for fp in range(head, FP):
                    schedule.append((fp, list(range(NCH))))
                    if backlog:
                        schedule.append((backlog.pop(0), rest))
                while backlog:
                    schedule.append((backlog.pop(0), rest))
            else:
                for fp in range(FP):
                    schedule.append((fp, list(range(NCH))))

            with nc.named_scope("gemm1"):
                for fp, chunks in schedule:
                    g1_entry(fp, chunks)

            # ---- GEMM2: k-outer over all chunks per weight tile
            with nc.named_scope("gemm2"):
                for m in range(MT):
                    w2s = wpool.tile([P, FP, P], bf16, tag="w2s", bufs=3)
                    nc.sync.dma_start(w2s[:], w2_d[m])
                    pys = [psum.tile([P, ws[c]], f32, tag="ps", name=f"py{c}")
                           for c in range(NCH)]
                    for k in range(FP):
                        for c in range(NCH):
                            nc.tensor.matmul(
                                pys[c], w2s[:, k],
                                act_sb[:, k, offs[c]:offs[c] + ws[c]],
                                start=(k == 0), stop=(k == FP - 1))
                    y = epool.tile([P, C], f32, tag="y", bufs=2)
                    for c in range(NCH):
                        ysl = y[:, offs[c]:offs[c] + ws[c]]
                        dst = yt_d[m][:, offs[c]:offs[c] + ws[c]]
                        # evac + store ride the same engine per chunk so the
                        # three chains drain in parallel at the kernel tail
                        if c % 2 == 0:
                            nc.scalar.activation(ysl, pys[c], Identity,
                                                 bias=b2_sb[:, m])
                            eng = nc.scalar if c == 0 else nc.sync
                        else:
                            nc.vector.tensor_scalar_add(ysl, pys[c],
                                                        b2_sb[:, m])
                            eng = nc.sync
                        eng.dma_start(dst, ysl)

    nc.compile()
    return nc


def kernel(hidden_states, expert_weights, w1, b1, w2, b2, top_experts):
    global last_exec_time_ns, last_results

    hidden_states = np.asarray(hidden_states)
    B, S, H = hidden_states.shape
    E, _, F2 = np.asarray(w1).shape
    F = F2 // 2
    topk = np.asarray(top_experts).shape[-1]
    N = B * S
    n_cores = 8
    assert E == n_cores, f"kernel assumes one expert per core, got E={E}"
    KT1 = H // P

    x = np.ascontiguousarray(hidden_states.reshape(N, H).astype(np.float32))
    te = np.asarray(top_experts).reshape(-1).astype(np.int64)
    ew = np.asarray(expert_weights).reshape(-1).astype(np.float32)

    counts = np.bincount(te, minlength=E)
    order = np.argsort(te, kind="stable")      # slot ids grouped by expert
    starts = np.zeros(E + 1, dtype=np.int64)
    starts[1:] = np.cumsum(counts)

    cmax = int(counts.max())
    ws = _widths(min(max(cmax, 1), C_MAX))
    cap = sum(ws)
    offs = [sum(ws[:i]) for i in range(len(ws))]
    n_waves = max(1, -(-cmax // cap))
    fpa = FPA if len(ws) >= 2 else 0

    key = (tuple(ws), fpa, H, F, n_cores)
    if key not in _cache:
        _cache[key] = _build(*key)
    nc = _cache[key]

    # per-expert constant inputs (weights pre-tiled to contiguous SBUF strips)
    FT, FP, MT = F2 // P, F // P, H // P
    const_maps = []
    for e in range(E):
        w1e = np.asarray(w1[e], dtype=np.float32).astype(ml_dtypes.bfloat16)
        w1t = np.ascontiguousarray(
            w1e.reshape(KT1, P, FT, P).transpose(2, 1, 0, 3))
        w2e = np.asarray(w2[e], dtype=np.float32).astype(ml_dtypes.bfloat16)
        w2t = np.ascontiguousarray(
            w2e.reshape(FP, P, MT, P).transpose(2, 1, 0, 3))
        const_maps.append({
            "w1t": w1t,
            "b1": np.ascontiguousarray(
                np.asarray(b1[e], dtype=np.float32).reshape(F2, 1)),
            "w2t": w2t,
            "b2": np.ascontiguousarray(
                np.asarray(b2[e], dtype=np.float32).reshape(H, 1)),
        })

    trace = os.environ.get("KERNEL_TRACE", "") == "1"
    if trace:
        _ensure_trace_hooks()
    out_pairs = np.zeros((N * topk, H), dtype=np.float32)
    last_results = []
    for w in range(n_waves):
        in_maps = []
        for e in range(E):
            lo = w * cap
            idx = order[starts[e] + lo: min(starts[e + 1], starts[e] + lo + cap)]
            toks = idx // topk
            xp = np.zeros((cap, H), dtype=ml_dtypes.bfloat16)
            if len(toks):
                xp[:len(toks)] = x[toks]
            # [cap, KT1, P] -> [P, KT1, cap] -> per-chunk contiguous strips
            arr = xp.reshape(cap, KT1, P).transpose(2, 1, 0)
            im = {f"xt{c}": np.ascontiguousarray(
                      arr[:, :, offs[c]:offs[c] + ws[c]]).reshape(P, -1)
                  for c in range(len(ws))}
            im.update(const_maps[e])
            in_maps.append(im)
        tmpdir = None
        if trace:
            import shutil
            tmpdir = f"/tmp/moe_trace_w{w}"
            shutil.rmtree(tmpdir, ignore_errors=True)
            os.makedirs(tmpdir, exist_ok=True)
        res = bass_utils.run_bass_kernel_spmd(
            nc, in_maps, core_ids=list(range(n_cores)), trace=trace,
            tmpdir=tmpdir)
        last_results.append(res)
        if trace:
            last_exec_time_ns = res.exec_time_ns
        for e in range(E):
            lo = w * cap
            idx = order[starts[e] + lo: min(starts[e + 1], starts[e] + lo + cap)]
            if len(idx):
                yt = np.asarray(res.results[e]["yt"],
                                dtype=np.float32).reshape(H, cap)
                out_pairs[idx] = yt[:, :len(idx)].T

    out = (out_pairs.reshape(N, topk, H) * ew.reshape(N, topk, 1)).sum(axis=1)
    return out.reshape(B, S, H).astype(np.float32)
